# revision 13
# baseline (speedup 1.0000x reference)
"""CGMM message-passing layer on 8 Trainium2 NeuronCores (Bass/Tile).

Strategy (graph/data parallel per the sharding hint):
  - Nodes are partitioned contiguously across the 8 cores (dst-sharding).
  - Per core, destination-sorted edges are bin-packed into "chunks" with
    <= CW output columns, <= 64 edge slots whose src is in the low half of
    the node table and <= 64 in the high half (dma_gather indices are
    int16, so the gather table is split into two halves).
  - Per 51-chunk panel, two dma_gather calls (one per half) pull the
    prev_h rows for all the panel's edges into SBUF in slot-major layout
    (row i -> partition i%128, slot i//128).  Chunk k's half0 slots land
    on partitions [(k%2)*64, +64) of pair-slot k//2; the host orders the
    half1 index stream so chunk k's half1 slots land on the complementary
    64 partitions.  Two K=64 matmuls per chunk against a 0/1
    segment-select matrix accumulate transposed per-node sums
    summedT[cg, node] in PSUM.
  - Downstream per 512-column panel, all in the transposed orientation:
      S   = Qbig^T @ summedT          (Qbig[lg, ig] = delta_g Q[i,l,g])
      Bx  = Bm^T  @ one_hot(x)        (Bm[m, ig]   = B[i,m,g])
      U   = S * Bx
      tot = Gsum^T @ U                (sum over i within ig)
      posteriorT = U * bcast(1/tot)
      likelihoodT = ln(tot / cnt)
  - The per-node mean (divide by degree) cancels in the posterior, so only
    the likelihood needs the count correction.
  - Host un-permutes the packed columns back to node order.
"""

from contextlib import ExitStack

import ml_dtypes
import numpy as np

import concourse.bacc as bacc
import concourse.bass as bass
import concourse.tile as tile
from concourse import mybir
from concourse.bass_utils import run_bass_kernel_spmd
from concourse.masks import make_identity

F32 = mybir.dt.float32
BF16 = mybir.dt.bfloat16
I16 = mybir.dt.int16

NCORES = 8
DEBUG = False
CW = 10          # output columns per chunk
PANEL = 500      # columns per downstream panel (50 chunks -> pairs never span)
CPP = PANEL // CW  # chunks per panel (50, even)
CPC = 8            # pairs per gather call (NH = 1024 <= SWDGE ring capacity)


def _pack_core(d0, d1):
    """Two-pointer packing of nodes (desc by total degree) into chunks of
    <= 64 half0 slots, <= 64 half1 slots, <= CW columns."""
    deg = d0 + d1
    order = np.argsort(-deg, kind="stable")
    lo, hi = 0, len(order) - 1
    bins = []
    while lo <= hi:
        s0, s1, cols, members = 0, 0, 0, []

        def fits(n):
            return s0 + d0[n] <= 64 and s1 + d1[n] <= 64

        while lo <= hi and cols < CW and fits(order[lo]):
            n = order[lo]
            members.append(n)
            s0 += d0[n]
            s1 += d1[n]
            lo += 1
            cols += 1
        while lo <= hi and cols < CW and fits(order[hi]):
            n = order[hi]
            members.append(n)
            s0 += d0[n]
            s1 += d1[n]
            hi -= 1
            cols += 1
        bins.append(members)
    return bins


def _wrap16(arr):
    """[NH] int16 position-order -> [128, NH//16] wrapped+replicated tile."""
    w = arr.reshape(-1, 16).T          # [16, NH//16]
    return np.ascontiguousarray(np.tile(w, (8, 1)))


def _preprocess(x, edge_index, prev_h):
    N, C, G = prev_h.shape
    CG = C * G
    assert CG == 128, "kernel assumes C*G == 128"
    assert N % NCORES == 0
    assert N + 1 <= 65535, "two int16 halves cannot address the table"
    NPC = N // NCORES
    H0 = (N + 1) // 2          # low-half rows; high half is [H0, N+1)

    dst = edge_index[0].astype(np.int64)
    src = edge_index[1].astype(np.int64)
    cnt = np.bincount(dst, minlength=N)

    eorder = np.lexsort((src, dst))
    src_sorted = src[eorder]
    d0 = np.bincount(dst[src < H0], minlength=N)
    d1 = cnt - d0
    assert d0.max() <= 64 and d1.max() <= 64, "per-half node degree > 64"
    rowptr = np.zeros(N + 1, np.int64)
    np.cumsum(cnt, out=rowptr[1:])

    per_core_bins = []
    for c in range(NCORES):
        sl = slice(c * NPC, (c + 1) * NPC)
        per_core_bins.append(_pack_core(d0[sl], d1[sl]))
    n_chunks = max(len(b) for b in per_core_bins)
    n_panels = -(-n_chunks // CPP)
    cols_pad = n_panels * PANEL

    # per-panel chunk counts; gather-call geometry (16 chunks = 8 pairs/call)
    panel_pc = [min(CPP, n_chunks - w * CPP) for w in range(n_panels)]
    n_pairs = -(-n_chunks // 2)
    n_calls = -(-n_pairs // CPC)
    call_pairs = [min(CPC, n_pairs - c * CPC) for c in range(n_calls)]
    call_icols = [128 * pr // 16 for pr in call_pairs]
    icol_off = np.concatenate([[0], np.cumsum(call_icols)]).astype(int)
    total_icols = int(icol_off[-1])

    idxa_all, idxb_all, sega_all, segb_all, xoh_all, cntb_all, colnode_all = \
        [], [], [], [], [], [], []
    for c in range(NCORES):
        bins = per_core_bins[c]
        fa = np.zeros((n_chunks, 64), np.int16)   # half0 idx per chunk
        fb = np.zeros((n_chunks, 64), np.int16)   # half1 idx per chunk
        sega = np.zeros((128, cols_pad), np.float32)
        segb = np.zeros((128, cols_pad), np.float32)
        xoh = np.zeros((32, cols_pad), np.float32)
        cntb = np.ones((8, cols_pad), np.float32)
        colnode = np.full(cols_pad, -1, np.int64)
        for t, members in enumerate(bins):
            w, k = divmod(t, CPP)
            col0 = w * PANEL + k * CW
            off0 = (t % 2) * 64
            off1 = 64 - off0
            s0 = s1 = 0
            for q, nl in enumerate(members):
                gn = c * NPC + nl
                a, b = d0[gn], d1[gn]
                srcs = src_sorted[rowptr[gn]:rowptr[gn + 1]]
                fa[t, s0:s0 + a] = srcs[:a]
                fb[t, s1:s1 + b] = srcs[a:] - H0
                sega[off0 + s0:off0 + s0 + a, col0 + q] = 1.0
                segb[off1 + s1:off1 + s1 + b, col0 + q] = 1.0
                xoh[x[gn], col0 + q] = 1.0
                cntb[:, col0 + q] = a + b
                colnode[col0 + q] = gn
                s0 += a
                s1 += b
        # build per-call wrapped index streams
        ia = np.zeros((128, total_icols), np.int16)
        ib = np.zeros((128, total_icols), np.int16)
        for c2 in range(n_calls):
            pr = call_pairs[c2]
            t0 = c2 * 2 * CPC
            pc2 = min(2 * CPC, n_chunks - t0)
            flat_a = np.zeros(128 * pr, np.int16)
            flat_b = np.zeros(128 * pr, np.int16)
            flat_a[:pc2 * 64] = fa[t0:t0 + pc2].reshape(-1)
            for k in range(2 * pr):
                kb = k + 1 if k % 2 == 0 else k - 1   # swapped pair order
                if kb < pc2:
                    flat_b[k * 64:(k + 1) * 64] = fb[t0 + kb]
            csl = slice(int(icol_off[c2]), int(icol_off[c2 + 1]))
            ia[:, csl] = _wrap16(flat_a)
            ib[:, csl] = _wrap16(flat_b)
        idxa_all.append(ia)
        idxb_all.append(ib)
        sega_all.append(sega.astype(ml_dtypes.bfloat16))
        segb_all.append(segb.astype(ml_dtypes.bfloat16))
        xoh_all.append(xoh.astype(ml_dtypes.bfloat16))
        cntb_all.append(cntb)
        colnode_all.append(colnode)

    table = np.concatenate(
        [prev_h.reshape(N, CG).astype(np.float32), np.ones((1, CG), np.float32)], axis=0
    )  # [N+1, 128]

    return dict(
        N=N, C=C, G=G, M=32, NPC=NPC, H0=H0, n_chunks=n_chunks, n_panels=n_panels,
        cols_pad=cols_pad, panel_pc=panel_pc, n_calls=n_calls,
        call_pairs=call_pairs, icol_off=icol_off, total_icols=total_icols,
        table0=np.ascontiguousarray(table[:H0]),
        table1=np.ascontiguousarray(table[H0:]),
        idxa=idxa_all, idxb=idxb_all, sega=sega_all, segb=segb_all, xoh=xoh_all,
        cntb=cntb_all, colnode=colnode_all, cnt=cnt,
    )


def _build_program(pp, lambda_Q, lambda_B):
    N, C, G, M, H0 = pp["N"], pp["C"], pp["G"], pp["M"], pp["H0"]
    CG = C * G
    n_chunks, n_panels, cols_pad = pp["n_chunks"], pp["n_panels"], pp["cols_pad"]
    total_icols, icol_off = pp["total_icols"], pp["icol_off"]

    # parameter layouts for on-device softmax
    lamq_lg = np.ascontiguousarray(
        np.transpose(lambda_Q, (1, 2, 0)).reshape(CG, C).astype(np.float32)
    )  # [l*G+g, i]
    lamb_ig = np.ascontiguousarray(
        np.transpose(lambda_B, (0, 2, 1)).reshape(CG, M).astype(np.float32)
    )  # [i*G+g, m]
    ig = np.arange(CG)
    gmask = (ig[:, None] % G == ig[None, :] % G).astype(np.float32)   # [lg, ig]
    gsum = (ig[:, None] % G == np.arange(G)[None, :]).astype(np.float32)  # [ig, 8]
    gsumt = np.ascontiguousarray(gsum.T)                                   # [8, ig]

    nc = bacc.Bacc("TRN2", target_bir_lowering=False, debug=False,
                   enable_asserts=False, num_devices=NCORES)

    t_table0 = nc.dram_tensor("table0", [H0, CG], F32, kind="ExternalInput")
    t_table1 = nc.dram_tensor("table1", [N + 1 - H0, CG], F32, kind="ExternalInput")
    t_idxa = nc.dram_tensor("idxa", [128, total_icols], I16, kind="ExternalInput")
    t_idxb = nc.dram_tensor("idxb", [128, total_icols], I16, kind="ExternalInput")
    t_sega = nc.dram_tensor("sega", [128, cols_pad], BF16, kind="ExternalInput")
    t_segb = nc.dram_tensor("segb", [128, cols_pad], BF16, kind="ExternalInput")
    t_xoh = nc.dram_tensor("xoh", [M, cols_pad], BF16, kind="ExternalInput")
    t_cntb = nc.dram_tensor("cntb", [G, cols_pad], F32, kind="ExternalInput")
    t_lamq = nc.dram_tensor("lamq", [CG, C], F32, kind="ExternalInput")
    t_lamb = nc.dram_tensor("lamb", [CG, M], F32, kind="ExternalInput")
    t_gmask = nc.dram_tensor("gmask", [CG, CG], F32, kind="ExternalInput")
    t_gsum = nc.dram_tensor("gsum", [CG, G], F32, kind="ExternalInput")
    t_gsumt = nc.dram_tensor("gsumt", [G, CG], F32, kind="ExternalInput")
    t_post = nc.dram_tensor("post_t", [CG, cols_pad], F32, kind="ExternalOutput")
    t_like = nc.dram_tensor("like_t", [G, cols_pad], F32, kind="ExternalOutput")
    if DEBUG:
        t_dbg_qbig = nc.dram_tensor("dbg_qbig", [CG, CG], F32, kind="ExternalOutput")
        t_dbg_bm = nc.dram_tensor("dbg_bm", [M, CG], F32, kind="ExternalOutput")
        t_dbg_sumt = nc.dram_tensor("dbg_sumt", [CG, cols_pad], F32, kind="ExternalOutput")
        t_dbg_msga = nc.dram_tensor("dbg_msga", [128, CPC * 128], F32, kind="ExternalOutput")
        t_dbg_msgb = nc.dram_tensor("dbg_msgb", [128, CPC * 128], F32, kind="ExternalOutput")
        t_dbg_u = nc.dram_tensor("dbg_u", [CG, cols_pad], F32, kind="ExternalOutput")

    with tile.TileContext(nc) as tc, ExitStack() as ctx:
        consts = ctx.enter_context(tc.tile_pool(name="consts", bufs=1))
        setup = ctx.enter_context(tc.tile_pool(name="setup", bufs=1))
        msgs = ctx.enter_context(tc.tile_pool(name="msgs", bufs=3))
        io = ctx.enter_context(tc.tile_pool(name="io", bufs=2))
        work = ctx.enter_context(tc.tile_pool(name="work", bufs=2))
        psum_win = ctx.enter_context(tc.tile_pool(name="pwin", bufs=2, space="PSUM"))
        psum_ds = ctx.enter_context(tc.tile_pool(name="pds", bufs=1, space="PSUM"))

        def softmax_rows(lam_dram, f):
            """softmax along free dim of a [128, f] tile; returns SBUF tile."""
            lam = setup.tile([CG, f], F32, name=f"lam{f}")
            nc.sync.dma_start(lam[:], lam_dram.ap())
            mx = setup.tile([CG, 1], F32, name=f"mx{f}")
            nc.vector.tensor_reduce(mx[:], lam[:], axis=mybir.AxisListType.X,
                                    op=mybir.AluOpType.max, negate=True)
            e = setup.tile([CG, f], F32, name=f"e{f}")
            nc.scalar.activation(e[:], lam[:], mybir.ActivationFunctionType.Exp,
                                 bias=mx[:], scale=1.0)
            s = setup.tile([CG, 1], F32, name=f"s{f}")
            nc.vector.tensor_reduce(s[:], e[:], axis=mybir.AxisListType.X,
                                    op=mybir.AluOpType.add)
            r = setup.tile([CG, 1], F32, name=f"r{f}")
            nc.vector.reciprocal(r[:], s[:])
            out = setup.tile([CG, f], F32, name=f"sm{f}")
            nc.vector.tensor_scalar_mul(out[:], e[:], r[:])
            return out

        # ---- setup: Qbig [lg, ig] and Bm [m, ig] ----
        qs = softmax_rows(t_lamq, C)                     # [128, 16]
        gmask_t = consts.tile([CG, CG], F32)
        nc.sync.dma_start(gmask_t[:], t_gmask.ap())
        qexp = setup.tile([CG, CG], F32)
        nc.vector.tensor_copy(qexp[:], qs[:, :, None].to_broadcast([CG, C, G]))
        qbig = consts.tile([CG, CG], F32)
        nc.vector.tensor_tensor(qbig[:], qexp[:], gmask_t[:],
                                op=mybir.AluOpType.mult)

        bs = softmax_rows(t_lamb, M)                     # [128, 32]
        ident = setup.tile([CG, CG], F32)
        make_identity(nc, ident[:])
        bm_p = psum_ds.tile([M, CG], F32, space="PSUM")
        nc.tensor.transpose(bm_p[:], bs[:], ident[:])
        bm = consts.tile([M, CG], F32)
        nc.vector.tensor_copy(bm[:], bm_p[:])

        if DEBUG:
            nc.sync.dma_start(t_dbg_qbig.ap(), qbig[:])
            nc.sync.dma_start(t_dbg_bm.ap(), bm[:])
        gsum_t = consts.tile([CG, G], F32)
        nc.sync.dma_start(gsum_t[:], t_gsum.ap())
        gsumt_t = consts.tile([G, CG], F32)
        nc.sync.dma_start(gsumt_t[:], t_gsumt.ap())

        # ---- main loop over panels / chunks; gather calls every 16 chunks ----
        msg_tiles = {}
        for w in range(n_panels):
            pc = pp["panel_pc"][w]
            pw = pc * CW
            csl = slice(w * PANEL, w * PANEL + pw)

            sega_raw = io.tile([128, pw], BF16, tag="segar")
            nc.sync.dma_start(sega_raw[:], t_sega.ap()[:, csl])
            sega_t = io.tile([128, pw], F32, tag="sega")
            nc.vector.tensor_copy(sega_t[:], sega_raw[:])
            segb_raw = io.tile([128, pw], BF16, tag="segbr")
            nc.sync.dma_start(segb_raw[:], t_segb.ap()[:, csl])
            segb_t = io.tile([128, pw], F32, tag="segb")
            nc.vector.tensor_copy(segb_t[:], segb_raw[:])
            xoh_raw = io.tile([M, pw], BF16, tag="xohr")
            nc.sync.dma_start(xoh_raw[:], t_xoh.ap()[:, csl])
            xoh_t = io.tile([M, pw], F32, tag="xoh")
            nc.vector.tensor_copy(xoh_t[:], xoh_raw[:])
            cnt_t = io.tile([G, pw], F32, tag="cnt")
            nc.sync.dma_start(cnt_t[:], t_cntb.ap()[:, csl])

            win = psum_win.tile([CG, pw], F32, space="PSUM", tag="win")
            nc.vector.memset(win[:], 0.0)
            for k in range(0, pc, 2):
                t = w * CPP + k
                cl = t // (2 * CPC)
                if cl not in msg_tiles:
                    pr = pp["call_pairs"][cl]
                    nh = 128 * pr
                    isl = slice(int(icol_off[cl]), int(icol_off[cl + 1]))
                    idxa_t = io.tile([128, nh // 16], I16, tag="idxa")
                    nc.sync.dma_start(idxa_t[:], t_idxa.ap()[:, isl])
                    idxb_t = io.tile([128, nh // 16], I16, tag="idxb")
                    nc.sync.dma_start(idxb_t[:], t_idxb.ap()[:, isl])
                    msga = msgs.tile([128, pr, 128], F32, tag="msga")
                    nc.gpsimd.dma_gather(msga[:], t_table0.ap(), idxa_t[:],
                                         nh, nh, CG)
                    msgb = msgs.tile([128, pr, 128], F32, tag="msgb")
                    nc.gpsimd.dma_gather(msgb[:], t_table1.ap(), idxb_t[:],
                                         nh, nh, CG)
                    if DEBUG and cl == 0:
                        nc.sync.dma_start(t_dbg_msga.ap()[:, :pr * 128], msga[:])
                        nc.sync.dma_start(t_dbg_msgb.ap()[:, :pr * 128], msgb[:])
                    msg_tiles = {cl: (msga, msgb)}
                msga, msgb = msg_tiles[cl]
                q0 = k * CW
                qn = min(2 * CW, pw - q0)
                s = (t // 2) % CPC
                nc.tensor.matmul(
                    win[:, q0:q0 + qn],
                    lhsT=msga[:, s, :],
                    rhs=sega_t[:, q0:q0 + qn],
                    start=False, stop=False, skip_group_check=True,
                )
                nc.tensor.matmul(
                    win[:, q0:q0 + qn],
                    lhsT=msgb[:, s, :],
                    rhs=segb_t[:, q0:q0 + qn],
                    start=False, stop=True, skip_group_check=True,
                )

            sumt = work.tile([CG, pw], F32, tag="sumt")
            nc.vector.tensor_copy(sumt[:], win[:])
            if DEBUG:
                nc.sync.dma_start(t_dbg_sumt.ap()[:, csl], sumt[:])

            s_p = psum_ds.tile([CG, pw], F32, space="PSUM", tag="s")
            nc.tensor.matmul(s_p[:], lhsT=qbig[:], rhs=sumt[:], start=True, stop=True)
            bx_p = psum_ds.tile([CG, pw], F32, space="PSUM", tag="bx")
            nc.tensor.matmul(bx_p[:], lhsT=bm[:], rhs=xoh_t[:], start=True, stop=True)

            bx_s = work.tile([CG, pw], F32, tag="bxs")
            nc.vector.tensor_copy(bx_s[:], bx_p[:])
            u = work.tile([CG, pw], F32, tag="u")
            nc.vector.tensor_tensor(u[:], s_p[:], bx_s[:], op=mybir.AluOpType.mult)
            if DEBUG:
                nc.sync.dma_start(t_dbg_u.ap()[:, csl], u[:])

            tot_p = psum_ds.tile([G, pw], F32, space="PSUM", tag="tot")
            nc.tensor.matmul(tot_p[:], lhsT=gsum_t[:], rhs=u[:], start=True, stop=True)

            rtot = work.tile([G, pw], F32, tag="rtot")
            nc.vector.reciprocal(rtot[:], tot_p[:])
            rb_p = psum_ds.tile([CG, pw], F32, space="PSUM", tag="rb")
            nc.tensor.matmul(rb_p[:], lhsT=gsumt_t[:], rhs=rtot[:], start=True, stop=True)

            post_t = work.tile([CG, pw], F32, tag="post")
            nc.vector.tensor_tensor(post_t[:], u[:], rb_p[:], op=mybir.AluOpType.mult)
            nc.sync.dma_start(t_post.ap()[:, csl], post_t[:])

            rcnt = work.tile([G, pw], F32, tag="rcnt")
            nc.vector.reciprocal(rcnt[:], cnt_t[:])
            totn = work.tile([G, pw], F32, tag="totn")
            nc.vector.tensor_tensor(totn[:], tot_p[:], rcnt[:], op=mybir.AluOpType.mult)
            like_t = work.tile([G, pw], F32, tag="like")
            nc.scalar.activation(like_t[:], totn[:], mybir.ActivationFunctionType.Ln)
            nc.sync.dma_start(t_like.ap()[:, csl], like_t[:])

    nc.compile()

    in_maps = []
    for c in range(NCORES):
        in_maps.append({
            "table0": pp["table0"],
            "table1": pp["table1"],
            "idxa": pp["idxa"][c],
            "idxb": pp["idxb"][c],
            "sega": np.asarray(pp["sega"][c]),
            "segb": np.asarray(pp["segb"][c]),
            "xoh": np.asarray(pp["xoh"][c]),
            "cntb": pp["cntb"][c],
            "lamq": lamq_lg,
            "lamb": lamb_ig,
            "gmask": gmask,
            "gsum": gsum,
            "gsumt": gsumt,
        })
    return nc, in_maps


def _postprocess(pp, results):
    N, C, G = pp["N"], pp["C"], pp["G"]
    likelihood = np.empty((N, G), np.float32)
    posterior = np.empty((N, C, G), np.float32)
    for c in range(NCORES):
        colnode = pp["colnode"][c]
        valid = colnode >= 0
        nodes = colnode[valid]
        pt = results[c]["post_t"]   # [128, cols_pad]
        lt = results[c]["like_t"]   # [8, cols_pad]
        posterior[nodes] = np.ascontiguousarray(pt[:, valid].T).reshape(-1, C, G)
        likelihood[nodes] = lt[:, valid].T
    return likelihood, posterior


def kernel(x, edge_index, prev_h, lambda_Q, lambda_B):
    x = np.asarray(x)
    edge_index = np.asarray(edge_index)
    prev_h = np.asarray(prev_h)
    lambda_Q = np.asarray(lambda_Q)
    lambda_B = np.asarray(lambda_B)

    pp = _preprocess(x, edge_index, prev_h)
    nc, in_maps = _build_program(pp, lambda_Q, lambda_B)
    res = run_bass_kernel_spmd(nc, in_maps, list(range(NCORES)))
    return _postprocess(pp, res.results)


# revision 14
# speedup vs baseline: 1.5646x; 1.5646x over previous
"""CGMM message-passing layer on 8 Trainium2 NeuronCores (Bass/Tile).

Strategy (graph/data parallel per the sharding hint):
  - Nodes are partitioned contiguously across the 8 cores (dst-sharding).
  - Per core, destination-sorted edges are bin-packed into "chunks" with
    <= CW output columns, <= 64 edge slots whose src is in the low half of
    the node table and <= 64 in the high half (dma_gather indices are
    int16, so the gather table is split into two halves).
  - Per 51-chunk panel, two dma_gather calls (one per half) pull the
    prev_h rows for all the panel's edges into SBUF in slot-major layout
    (row i -> partition i%128, slot i//128).  Chunk k's half0 slots land
    on partitions [(k%2)*64, +64) of pair-slot k//2; the host orders the
    half1 index stream so chunk k's half1 slots land on the complementary
    64 partitions.  Two K=64 matmuls per chunk against a 0/1
    segment-select matrix accumulate transposed per-node sums
    summedT[cg, node] in PSUM.
  - Downstream per 512-column panel, all in the transposed orientation:
      S   = Qbig^T @ summedT          (Qbig[lg, ig] = delta_g Q[i,l,g])
      Bx  = Bm^T  @ one_hot(x)        (Bm[m, ig]   = B[i,m,g])
      U   = S * Bx
      tot = Gsum^T @ U                (sum over i within ig)
      posteriorT = U * bcast(1/tot)
      likelihoodT = ln(tot / cnt)
  - The per-node mean (divide by degree) cancels in the posterior, so only
    the likelihood needs the count correction.
  - Host un-permutes the packed columns back to node order.
"""

from contextlib import ExitStack

import ml_dtypes
import numpy as np

import concourse.bacc as bacc
import concourse.bass as bass
import concourse.tile as tile
from concourse import mybir
from concourse.bass_utils import run_bass_kernel_spmd
from concourse.masks import make_identity

F32 = mybir.dt.float32
BF16 = mybir.dt.bfloat16
I16 = mybir.dt.int16

NCORES = 8
DEBUG = False
CW = 10          # output columns per chunk
PANEL = 500      # columns per downstream panel (50 chunks -> pairs never span)
CPP = PANEL // CW  # chunks per panel (50, even)
CPC = 8            # pairs per gather call (NH = 1024 <= SWDGE ring capacity)


def _pack_core(d0, d1):
    """Two-pointer packing of nodes (desc by total degree) into chunks of
    <= 64 half0 slots, <= 64 half1 slots, <= CW columns."""
    deg = d0 + d1
    order = np.argsort(-deg, kind="stable")
    lo, hi = 0, len(order) - 1
    bins = []
    while lo <= hi:
        s0, s1, cols, members = 0, 0, 0, []

        def fits(n):
            return s0 + d0[n] <= 64 and s1 + d1[n] <= 64

        while lo <= hi and cols < CW and fits(order[lo]):
            n = order[lo]
            members.append(n)
            s0 += d0[n]
            s1 += d1[n]
            lo += 1
            cols += 1
        while lo <= hi and cols < CW and fits(order[hi]):
            n = order[hi]
            members.append(n)
            s0 += d0[n]
            s1 += d1[n]
            hi -= 1
            cols += 1
        bins.append(members)
    return bins


def _wrap16(arr):
    """[NH] int16 position-order -> [128, NH//16] wrapped+replicated tile."""
    w = arr.reshape(-1, 16).T          # [16, NH//16]
    return np.ascontiguousarray(np.tile(w, (8, 1)))


def _preprocess(x, edge_index, prev_h):
    N, C, G = prev_h.shape
    CG = C * G
    assert CG == 128, "kernel assumes C*G == 128"
    assert N % NCORES == 0
    assert N + 1 <= 65535, "two int16 halves cannot address the table"
    NPC = N // NCORES
    H0 = (N + 1) // 2          # low-half rows; high half is [H0, N+1)

    dst = edge_index[0].astype(np.int64)
    src = edge_index[1].astype(np.int64)
    cnt = np.bincount(dst, minlength=N)

    eorder = np.lexsort((src, dst))
    src_sorted = src[eorder]
    d0 = np.bincount(dst[src < H0], minlength=N)
    d1 = cnt - d0
    assert d0.max() <= 64 and d1.max() <= 64, "per-half node degree > 64"
    rowptr = np.zeros(N + 1, np.int64)
    np.cumsum(cnt, out=rowptr[1:])

    per_core_bins = []
    for c in range(NCORES):
        sl = slice(c * NPC, (c + 1) * NPC)
        per_core_bins.append(_pack_core(d0[sl], d1[sl]))
    n_chunks = max(len(b) for b in per_core_bins)
    n_panels = -(-n_chunks // CPP)
    cols_pad = n_panels * PANEL

    # per-panel chunk counts; gather-call geometry (16 chunks = 8 pairs/call)
    panel_pc = [min(CPP, n_chunks - w * CPP) for w in range(n_panels)]
    n_pairs = -(-n_chunks // 2)
    n_calls = -(-n_pairs // CPC)
    call_pairs = [min(CPC, n_pairs - c * CPC) for c in range(n_calls)]
    call_icols = [128 * pr // 16 for pr in call_pairs]
    icol_off = np.concatenate([[0], np.cumsum(call_icols)]).astype(int)
    total_icols = int(icol_off[-1])

    idxa_all, idxb_all, sega_all, segb_all, xoh_all, cntb_all, colnode_all = \
        [], [], [], [], [], [], []
    for c in range(NCORES):
        bins = per_core_bins[c]
        fa = np.zeros((n_chunks, 64), np.int16)   # half0 idx per chunk
        fb = np.zeros((n_chunks, 64), np.int16)   # half1 idx per chunk
        sega = np.zeros((128, cols_pad), np.float32)
        segb = np.zeros((128, cols_pad), np.float32)
        xoh = np.zeros((32, cols_pad), np.float32)
        cntb = np.ones((8, cols_pad), np.float32)
        colnode = np.full(cols_pad, -1, np.int64)
        for t, members in enumerate(bins):
            w, k = divmod(t, CPP)
            col0 = w * PANEL + k * CW
            off0 = (t % 2) * 64
            off1 = 64 - off0
            s0 = s1 = 0
            for q, nl in enumerate(members):
                gn = c * NPC + nl
                a, b = d0[gn], d1[gn]
                srcs = src_sorted[rowptr[gn]:rowptr[gn + 1]]
                fa[t, s0:s0 + a] = srcs[:a]
                fb[t, s1:s1 + b] = srcs[a:] - H0
                sega[off0 + s0:off0 + s0 + a, col0 + q] = 1.0
                segb[off1 + s1:off1 + s1 + b, col0 + q] = 1.0
                xoh[x[gn], col0 + q] = 1.0
                cntb[:, col0 + q] = 1.0 / (a + b) if a + b else 1.0
                colnode[col0 + q] = gn
                s0 += a
                s1 += b
        # build per-call wrapped index streams
        ia = np.zeros((128, total_icols), np.int16)
        ib = np.zeros((128, total_icols), np.int16)
        for c2 in range(n_calls):
            pr = call_pairs[c2]
            t0 = c2 * 2 * CPC
            pc2 = min(2 * CPC, n_chunks - t0)
            flat_a = np.zeros(128 * pr, np.int16)
            flat_b = np.zeros(128 * pr, np.int16)
            flat_a[:pc2 * 64] = fa[t0:t0 + pc2].reshape(-1)
            for k in range(2 * pr):
                kb = k + 1 if k % 2 == 0 else k - 1   # swapped pair order
                if kb < pc2:
                    flat_b[k * 64:(k + 1) * 64] = fb[t0 + kb]
            csl = slice(int(icol_off[c2]), int(icol_off[c2 + 1]))
            ia[:, csl] = _wrap16(flat_a)
            ib[:, csl] = _wrap16(flat_b)
        idxa_all.append(ia)
        idxb_all.append(ib)
        sega_all.append(sega)
        segb_all.append(segb)
        xoh_all.append(xoh)
        cntb_all.append(cntb)
        colnode_all.append(colnode)

    table = np.concatenate(
        [prev_h.reshape(N, CG).astype(np.float32), np.ones((1, CG), np.float32)], axis=0
    )  # [N+1, 128]

    return dict(
        N=N, C=C, G=G, M=32, NPC=NPC, H0=H0, n_chunks=n_chunks, n_panels=n_panels,
        cols_pad=cols_pad, panel_pc=panel_pc, n_calls=n_calls,
        call_pairs=call_pairs, icol_off=icol_off, total_icols=total_icols,
        table0=np.ascontiguousarray(table[:H0]),
        table1=np.ascontiguousarray(table[H0:]),
        idxa=idxa_all, idxb=idxb_all, sega=sega_all, segb=segb_all, xoh=xoh_all,
        cntb=cntb_all, colnode=colnode_all, cnt=cnt,
    )


def _build_program(pp, lambda_Q, lambda_B):
    N, C, G, M, H0 = pp["N"], pp["C"], pp["G"], pp["M"], pp["H0"]
    CG = C * G
    n_chunks, n_panels, cols_pad = pp["n_chunks"], pp["n_panels"], pp["cols_pad"]
    total_icols, icol_off = pp["total_icols"], pp["icol_off"]

    # parameter layouts for on-device softmax
    lamq_lg = np.ascontiguousarray(
        np.transpose(lambda_Q, (1, 2, 0)).reshape(CG, C).astype(np.float32)
    )  # [l*G+g, i]
    lamb_ig = np.ascontiguousarray(
        np.transpose(lambda_B, (0, 2, 1)).reshape(CG, M).astype(np.float32)
    )  # [i*G+g, m]
    ig = np.arange(CG)
    gmask = (ig[:, None] % G == ig[None, :] % G).astype(np.float32)   # [lg, ig]
    gsum = (ig[:, None] % G == np.arange(G)[None, :]).astype(np.float32)  # [ig, 8]
    gsumt = np.ascontiguousarray(gsum.T)                                   # [8, ig]

    nc = bacc.Bacc("TRN2", target_bir_lowering=False, debug=False,
                   enable_asserts=False, num_devices=NCORES, num_swdge_queues=4)

    t_table0 = nc.dram_tensor("table0", [H0, CG], F32, kind="ExternalInput")
    t_table1 = nc.dram_tensor("table1", [N + 1 - H0, CG], F32, kind="ExternalInput")
    t_idxa = nc.dram_tensor("idxa", [128, total_icols], I16, kind="ExternalInput")
    t_idxb = nc.dram_tensor("idxb", [128, total_icols], I16, kind="ExternalInput")
    t_sega = nc.dram_tensor("sega", [128, cols_pad], F32, kind="ExternalInput")
    t_segb = nc.dram_tensor("segb", [128, cols_pad], F32, kind="ExternalInput")
    t_xoh = nc.dram_tensor("xoh", [M, cols_pad], F32, kind="ExternalInput")
    t_cntb = nc.dram_tensor("cntb", [G, cols_pad], F32, kind="ExternalInput")
    t_lamq = nc.dram_tensor("lamq", [CG, C], F32, kind="ExternalInput")
    t_lamb = nc.dram_tensor("lamb", [CG, M], F32, kind="ExternalInput")
    t_gmask = nc.dram_tensor("gmask", [CG, CG], F32, kind="ExternalInput")
    t_gsum = nc.dram_tensor("gsum", [CG, G], F32, kind="ExternalInput")
    t_gsumt = nc.dram_tensor("gsumt", [G, CG], F32, kind="ExternalInput")
    t_post = nc.dram_tensor("post_t", [CG, cols_pad], F32, kind="ExternalOutput")
    t_like = nc.dram_tensor("like_t", [G, cols_pad], F32, kind="ExternalOutput")
    if DEBUG:
        t_dbg_qbig = nc.dram_tensor("dbg_qbig", [CG, CG], F32, kind="ExternalOutput")
        t_dbg_bm = nc.dram_tensor("dbg_bm", [M, CG], F32, kind="ExternalOutput")
        t_dbg_sumt = nc.dram_tensor("dbg_sumt", [CG, cols_pad], F32, kind="ExternalOutput")
        t_dbg_msga = nc.dram_tensor("dbg_msga", [128, CPC * 128], F32, kind="ExternalOutput")
        t_dbg_msgb = nc.dram_tensor("dbg_msgb", [128, CPC * 128], F32, kind="ExternalOutput")
        t_dbg_u = nc.dram_tensor("dbg_u", [CG, cols_pad], F32, kind="ExternalOutput")

    with tile.TileContext(nc) as tc, ExitStack() as ctx:
        consts = ctx.enter_context(tc.tile_pool(name="consts", bufs=1))
        setup = ctx.enter_context(tc.tile_pool(name="setup", bufs=1))
        msgs = ctx.enter_context(tc.tile_pool(name="msgs", bufs=3))
        io = ctx.enter_context(tc.tile_pool(name="io", bufs=2))
        work = ctx.enter_context(tc.tile_pool(name="work", bufs=2))
        psum_win = ctx.enter_context(tc.tile_pool(name="pwin", bufs=2, space="PSUM"))
        psum_ds = ctx.enter_context(tc.tile_pool(name="pds", bufs=1, space="PSUM"))

        def softmax_rows(lam_dram, f):
            """softmax along free dim of a [128, f] tile; returns SBUF tile."""
            lam = setup.tile([CG, f], F32, name=f"lam{f}")
            nc.sync.dma_start(lam[:], lam_dram.ap())
            mx = setup.tile([CG, 1], F32, name=f"mx{f}")
            nc.vector.tensor_reduce(mx[:], lam[:], axis=mybir.AxisListType.X,
                                    op=mybir.AluOpType.max, negate=True)
            e = setup.tile([CG, f], F32, name=f"e{f}")
            nc.scalar.activation(e[:], lam[:], mybir.ActivationFunctionType.Exp,
                                 bias=mx[:], scale=1.0)
            s = setup.tile([CG, 1], F32, name=f"s{f}")
            nc.vector.tensor_reduce(s[:], e[:], axis=mybir.AxisListType.X,
                                    op=mybir.AluOpType.add)
            r = setup.tile([CG, 1], F32, name=f"r{f}")
            nc.vector.reciprocal(r[:], s[:])
            out = setup.tile([CG, f], F32, name=f"sm{f}")
            nc.vector.tensor_scalar_mul(out[:], e[:], r[:])
            return out

        # ---- setup: Qbig [lg, ig] and Bm [m, ig] ----
        qs = softmax_rows(t_lamq, C)                     # [128, 16]
        gmask_t = consts.tile([CG, CG], F32)
        nc.sync.dma_start(gmask_t[:], t_gmask.ap())
        qexp = setup.tile([CG, CG], F32)
        nc.vector.tensor_copy(qexp[:], qs[:, :, None].to_broadcast([CG, C, G]))
        qbig = consts.tile([CG, CG], F32)
        nc.vector.tensor_tensor(qbig[:], qexp[:], gmask_t[:],
                                op=mybir.AluOpType.mult)

        bs = softmax_rows(t_lamb, M)                     # [128, 32]
        ident = setup.tile([CG, CG], F32)
        make_identity(nc, ident[:])
        bm_p = psum_ds.tile([M, CG], F32, space="PSUM")
        nc.tensor.transpose(bm_p[:], bs[:], ident[:])
        bm = consts.tile([M, CG], F32)
        nc.vector.tensor_copy(bm[:], bm_p[:])

        if DEBUG:
            nc.sync.dma_start(t_dbg_qbig.ap(), qbig[:])
            nc.sync.dma_start(t_dbg_bm.ap(), bm[:])
        gsum_t = consts.tile([CG, G], F32)
        nc.sync.dma_start(gsum_t[:], t_gsum.ap())
        gsumt_t = consts.tile([G, CG], F32)
        nc.sync.dma_start(gsumt_t[:], t_gsumt.ap())

        # ---- main loop over panels / chunks; gather calls every 16 chunks ----
        msg_tiles = {}
        for w in range(n_panels):
            pc = pp["panel_pc"][w]
            pw = pc * CW
            csl = slice(w * PANEL, w * PANEL + pw)

            sega_t = io.tile([128, pw], F32, tag="sega")
            nc.sync.dma_start(sega_t[:], t_sega.ap()[:, csl])
            segb_t = io.tile([128, pw], F32, tag="segb")
            nc.sync.dma_start(segb_t[:], t_segb.ap()[:, csl])
            xoh_t = io.tile([M, pw], F32, tag="xoh")
            nc.sync.dma_start(xoh_t[:], t_xoh.ap()[:, csl])
            cnt_t = io.tile([G, pw], F32, tag="cnt")
            nc.sync.dma_start(cnt_t[:], t_cntb.ap()[:, csl])

            win = psum_win.tile([CG, pw], F32, space="PSUM", tag="win")
            nc.vector.memset(win[:], 0.0)
            for k in range(0, pc, 2):
                t = w * CPP + k
                cl = t // (2 * CPC)
                if cl not in msg_tiles:
                    pr = pp["call_pairs"][cl]
                    nh = 128 * pr
                    isl = slice(int(icol_off[cl]), int(icol_off[cl + 1]))
                    idxa_t = io.tile([128, nh // 16], I16, tag="idxa")
                    nc.sync.dma_start(idxa_t[:], t_idxa.ap()[:, isl])
                    idxb_t = io.tile([128, nh // 16], I16, tag="idxb")
                    nc.sync.dma_start(idxb_t[:], t_idxb.ap()[:, isl])
                    msga = msgs.tile([128, pr, 128], F32, tag="msga")
                    nc.gpsimd.dma_gather(msga[:], t_table0.ap(), idxa_t[:],
                                         nh, nh, CG, queue_num=(2 * cl) % 4)
                    msgb = msgs.tile([128, pr, 128], F32, tag="msgb")
                    nc.gpsimd.dma_gather(msgb[:], t_table1.ap(), idxb_t[:],
                                         nh, nh, CG, queue_num=(2 * cl + 1) % 4)
                    if DEBUG and cl == 0:
                        nc.sync.dma_start(t_dbg_msga.ap()[:, :pr * 128], msga[:])
                        nc.sync.dma_start(t_dbg_msgb.ap()[:, :pr * 128], msgb[:])
                    msg_tiles = {cl: (msga, msgb)}
                msga, msgb = msg_tiles[cl]
                q0 = k * CW
                qn = min(2 * CW, pw - q0)
                s = (t // 2) % CPC
                nc.tensor.matmul(
                    win[:, q0:q0 + qn],
                    lhsT=msga[:, s, :],
                    rhs=sega_t[:, q0:q0 + qn],
                    start=False, stop=False, skip_group_check=True,
                )
                nc.tensor.matmul(
                    win[:, q0:q0 + qn],
                    lhsT=msgb[:, s, :],
                    rhs=segb_t[:, q0:q0 + qn],
                    start=False, stop=True, skip_group_check=True,
                )

            sumt = work.tile([CG, pw], F32, tag="sumt")
            nc.vector.tensor_copy(sumt[:], win[:])
            if DEBUG:
                nc.sync.dma_start(t_dbg_sumt.ap()[:, csl], sumt[:])

            s_p = psum_ds.tile([CG, pw], F32, space="PSUM", tag="s")
            nc.tensor.matmul(s_p[:], lhsT=qbig[:], rhs=sumt[:], start=True, stop=True)
            bx_p = psum_ds.tile([CG, pw], F32, space="PSUM", tag="bx")
            nc.tensor.matmul(bx_p[:], lhsT=bm[:], rhs=xoh_t[:], start=True, stop=True)

            bx_s = work.tile([CG, pw], F32, tag="bxs")
            nc.vector.tensor_copy(bx_s[:], bx_p[:])
            u = work.tile([CG, pw], F32, tag="u")
            nc.vector.tensor_tensor(u[:], s_p[:], bx_s[:], op=mybir.AluOpType.mult)
            if DEBUG:
                nc.sync.dma_start(t_dbg_u.ap()[:, csl], u[:])

            tot_p = psum_ds.tile([G, pw], F32, space="PSUM", tag="tot")
            nc.tensor.matmul(tot_p[:], lhsT=gsum_t[:], rhs=u[:], start=True, stop=True)

            rtot = work.tile([G, pw], F32, tag="rtot")
            nc.vector.reciprocal(rtot[:], tot_p[:])
            rb_p = psum_ds.tile([CG, pw], F32, space="PSUM", tag="rb")
            nc.tensor.matmul(rb_p[:], lhsT=gsumt_t[:], rhs=rtot[:], start=True, stop=True)

            post_t = work.tile([CG, pw], F32, tag="post")
            nc.vector.tensor_tensor(post_t[:], u[:], rb_p[:], op=mybir.AluOpType.mult)
            nc.sync.dma_start(t_post.ap()[:, csl], post_t[:])

            totn = work.tile([G, pw], F32, tag="totn")
            nc.vector.tensor_tensor(totn[:], tot_p[:], cnt_t[:], op=mybir.AluOpType.mult)
            like_t = work.tile([G, pw], F32, tag="like")
            nc.scalar.activation(like_t[:], totn[:], mybir.ActivationFunctionType.Ln)
            nc.sync.dma_start(t_like.ap()[:, csl], like_t[:])

    nc.compile()

    in_maps = []
    for c in range(NCORES):
        in_maps.append({
            "table0": pp["table0"],
            "table1": pp["table1"],
            "idxa": pp["idxa"][c],
            "idxb": pp["idxb"][c],
            "sega": np.asarray(pp["sega"][c]),
            "segb": np.asarray(pp["segb"][c]),
            "xoh": np.asarray(pp["xoh"][c]),
            "cntb": pp["cntb"][c],
            "lamq": lamq_lg,
            "lamb": lamb_ig,
            "gmask": gmask,
            "gsum": gsum,
            "gsumt": gsumt,
        })
    return nc, in_maps


def _postprocess(pp, results):
    N, C, G = pp["N"], pp["C"], pp["G"]
    likelihood = np.empty((N, G), np.float32)
    posterior = np.empty((N, C, G), np.float32)
    for c in range(NCORES):
        colnode = pp["colnode"][c]
        valid = colnode >= 0
        nodes = colnode[valid]
        pt = results[c]["post_t"]   # [128, cols_pad]
        lt = results[c]["like_t"]   # [8, cols_pad]
        posterior[nodes] = np.ascontiguousarray(pt[:, valid].T).reshape(-1, C, G)
        likelihood[nodes] = lt[:, valid].T
    return likelihood, posterior


def kernel(x, edge_index, prev_h, lambda_Q, lambda_B):
    x = np.asarray(x)
    edge_index = np.asarray(edge_index)
    prev_h = np.asarray(prev_h)
    lambda_Q = np.asarray(lambda_Q)
    lambda_B = np.asarray(lambda_B)

    pp = _preprocess(x, edge_index, prev_h)
    nc, in_maps = _build_program(pp, lambda_Q, lambda_B)
    res = run_bass_kernel_spmd(nc, in_maps, list(range(NCORES)))
    return _postprocess(pp, res.results)


# revision 16
# speedup vs baseline: 1.9674x; 1.2575x over previous
"""CGMM message-passing layer on 8 Trainium2 NeuronCores (Bass/Tile).

Strategy (graph/data parallel per the sharding hint):
  - Nodes are partitioned contiguously across the 8 cores (dst-sharding).
  - Per core, destination-sorted edges are bin-packed into "chunks" with
    <= CW output columns, <= 64 edge slots whose src is in the low half of
    the node table and <= 64 in the high half (dma_gather indices are
    int16, so the gather table is split into two halves).
  - Per 51-chunk panel, two dma_gather calls (one per half) pull the
    prev_h rows for all the panel's edges into SBUF in slot-major layout
    (row i -> partition i%128, slot i//128).  Chunk k's half0 slots land
    on partitions [(k%2)*64, +64) of pair-slot k//2; the host orders the
    half1 index stream so chunk k's half1 slots land on the complementary
    64 partitions.  Two K=64 matmuls per chunk against a 0/1
    segment-select matrix accumulate transposed per-node sums
    summedT[cg, node] in PSUM.
  - Downstream per 512-column panel, all in the transposed orientation:
      S   = Qbig^T @ summedT          (Qbig[lg, ig] = delta_g Q[i,l,g])
      Bx  = Bm^T  @ one_hot(x)        (Bm[m, ig]   = B[i,m,g])
      U   = S * Bx
      tot = Gsum^T @ U                (sum over i within ig)
      posteriorT = U * bcast(1/tot)
      likelihoodT = ln(tot / cnt)
  - The per-node mean (divide by degree) cancels in the posterior, so only
    the likelihood needs the count correction.
  - Host un-permutes the packed columns back to node order.
"""

from contextlib import ExitStack

import ml_dtypes
import numpy as np

import concourse.bacc as bacc
import concourse.bass as bass
import concourse.tile as tile
from concourse import mybir
from concourse.bass_utils import run_bass_kernel_spmd
from concourse.masks import make_identity

F32 = mybir.dt.float32
F32R = mybir.dt.float32r
BF16 = mybir.dt.bfloat16
I16 = mybir.dt.int16

NCORES = 8
DEBUG = False
SEG_BF16 = True    # bf16 gather table + segment matmuls
DS_F32R = False    # fp32r downstream matmuls (verifier needs rounded producers)
CW = 10          # output columns per chunk
PANEL = 500      # columns per downstream panel (50 chunks -> pairs never span)
CPP = PANEL // CW  # chunks per panel (50, even)
CPC = 8            # pairs per gather call (NH = 1024 <= SWDGE ring capacity)


def _pack_core(d0, d1):
    """Two-pointer packing of nodes (desc by total degree) into chunks of
    <= 64 half0 slots, <= 64 half1 slots, <= CW columns."""
    deg = d0 + d1
    order = np.argsort(-deg, kind="stable")
    lo, hi = 0, len(order) - 1
    bins = []
    while lo <= hi:
        s0, s1, cols, members = 0, 0, 0, []

        def fits(n):
            return s0 + d0[n] <= 64 and s1 + d1[n] <= 64

        while lo <= hi and cols < CW and fits(order[lo]):
            n = order[lo]
            members.append(n)
            s0 += d0[n]
            s1 += d1[n]
            lo += 1
            cols += 1
        while lo <= hi and cols < CW and fits(order[hi]):
            n = order[hi]
            members.append(n)
            s0 += d0[n]
            s1 += d1[n]
            hi -= 1
            cols += 1
        bins.append(members)
    return bins


def _wrap16(arr):
    """[NH] int16 position-order -> [128, NH//16] wrapped+replicated tile."""
    w = arr.reshape(-1, 16).T          # [16, NH//16]
    return np.ascontiguousarray(np.tile(w, (8, 1)))


def _preprocess(x, edge_index, prev_h):
    N, C, G = prev_h.shape
    CG = C * G
    assert CG == 128, "kernel assumes C*G == 128"
    assert N % NCORES == 0
    assert N + 1 <= 65535, "two int16 halves cannot address the table"
    NPC = N // NCORES
    H0 = (N + 1) // 2          # low-half rows; high half is [H0, N+1)

    dst = edge_index[0].astype(np.int64)
    src = edge_index[1].astype(np.int64)
    cnt = np.bincount(dst, minlength=N)

    eorder = np.lexsort((src, dst))
    src_sorted = src[eorder]
    d0 = np.bincount(dst[src < H0], minlength=N)
    d1 = cnt - d0
    assert d0.max() <= 64 and d1.max() <= 64, "per-half node degree > 64"
    rowptr = np.zeros(N + 1, np.int64)
    np.cumsum(cnt, out=rowptr[1:])

    per_core_bins = []
    for c in range(NCORES):
        sl = slice(c * NPC, (c + 1) * NPC)
        per_core_bins.append(_pack_core(d0[sl], d1[sl]))
    n_chunks = max(len(b) for b in per_core_bins)
    n_panels = -(-n_chunks // CPP)
    cols_pad = n_panels * PANEL

    # per-panel chunk counts; gather-call geometry (16 chunks = 8 pairs/call)
    panel_pc = [min(CPP, n_chunks - w * CPP) for w in range(n_panels)]
    n_pairs = -(-n_chunks // 2)
    n_calls = -(-n_pairs // CPC)
    call_pairs = [min(CPC, n_pairs - c * CPC) for c in range(n_calls)]
    call_icols = [128 * pr // 16 for pr in call_pairs]
    icol_off = np.concatenate([[0], np.cumsum(call_icols)]).astype(int)
    total_icols = int(icol_off[-1])

    tdt = ml_dtypes.bfloat16 if SEG_BF16 else np.float32
    idxa_all, idxb_all, sega_all, segb_all, xoh_all, cntb_all, colnode_all = \
        [], [], [], [], [], [], []
    for c in range(NCORES):
        bins = per_core_bins[c]
        fa = np.zeros((n_chunks, 64), np.int16)   # half0 idx per chunk
        fb = np.zeros((n_chunks, 64), np.int16)   # half1 idx per chunk
        sega = np.zeros((128, cols_pad), np.float32)
        segb = np.zeros((128, cols_pad), np.float32)
        xoh = np.zeros((32, cols_pad), np.float32)
        cntb = np.ones((8, cols_pad), np.float32)
        colnode = np.full(cols_pad, -1, np.int64)
        for t, members in enumerate(bins):
            w, k = divmod(t, CPP)
            col0 = w * PANEL + k * CW
            off0 = (t % 2) * 64
            off1 = 64 - off0
            s0 = s1 = 0
            for q, nl in enumerate(members):
                gn = c * NPC + nl
                a, b = d0[gn], d1[gn]
                srcs = src_sorted[rowptr[gn]:rowptr[gn + 1]]
                fa[t, s0:s0 + a] = srcs[:a]
                fb[t, s1:s1 + b] = srcs[a:] - H0
                sega[off0 + s0:off0 + s0 + a, col0 + q] = 1.0
                segb[off1 + s1:off1 + s1 + b, col0 + q] = 1.0
                xoh[x[gn], col0 + q] = 1.0
                cntb[:, col0 + q] = 1.0 / (a + b) if a + b else 1.0
                colnode[col0 + q] = gn
                s0 += a
                s1 += b
        # build per-call wrapped index streams
        ia = np.zeros((128, total_icols), np.int16)
        ib = np.zeros((128, total_icols), np.int16)
        for c2 in range(n_calls):
            pr = call_pairs[c2]
            t0 = c2 * 2 * CPC
            pc2 = min(2 * CPC, n_chunks - t0)
            flat_a = np.zeros(128 * pr, np.int16)
            flat_b = np.zeros(128 * pr, np.int16)
            flat_a[:pc2 * 64] = fa[t0:t0 + pc2].reshape(-1)
            for k in range(2 * pr):
                kb = k + 1 if k % 2 == 0 else k - 1   # swapped pair order
                if kb < pc2:
                    flat_b[k * 64:(k + 1) * 64] = fb[t0 + kb]
            csl = slice(int(icol_off[c2]), int(icol_off[c2 + 1]))
            ia[:, csl] = _wrap16(flat_a)
            ib[:, csl] = _wrap16(flat_b)
        idxa_all.append(ia)
        idxb_all.append(ib)
        sega_all.append(sega.astype(tdt))
        segb_all.append(segb.astype(tdt))
        xoh_all.append(xoh)
        cntb_all.append(cntb)
        colnode_all.append(colnode)

    table = np.concatenate(
        [prev_h.reshape(N, CG).astype(tdt), np.ones((1, CG), tdt)], axis=0
    )  # [N+1, 128]

    return dict(
        N=N, C=C, G=G, M=32, NPC=NPC, H0=H0, n_chunks=n_chunks, n_panels=n_panels,
        cols_pad=cols_pad, panel_pc=panel_pc, n_calls=n_calls,
        call_pairs=call_pairs, icol_off=icol_off, total_icols=total_icols,
        table0=np.ascontiguousarray(table[:H0]),
        table1=np.ascontiguousarray(table[H0:]),
        idxa=idxa_all, idxb=idxb_all, sega=sega_all, segb=segb_all, xoh=xoh_all,
        cntb=cntb_all, colnode=colnode_all, cnt=cnt,
    )


def _build_program(pp, lambda_Q, lambda_B):
    N, C, G, M, H0 = pp["N"], pp["C"], pp["G"], pp["M"], pp["H0"]
    CG = C * G
    n_chunks, n_panels, cols_pad = pp["n_chunks"], pp["n_panels"], pp["cols_pad"]
    total_icols, icol_off = pp["total_icols"], pp["icol_off"]

    # parameter layouts for on-device softmax
    lamq_lg = np.ascontiguousarray(
        np.transpose(lambda_Q, (1, 2, 0)).reshape(CG, C).astype(np.float32)
    )  # [l*G+g, i]
    lamb_ig = np.ascontiguousarray(
        np.transpose(lambda_B, (0, 2, 1)).reshape(CG, M).astype(np.float32)
    )  # [i*G+g, m]
    ig = np.arange(CG)
    gmask = (ig[:, None] % G == ig[None, :] % G).astype(np.float32)   # [lg, ig]
    gsum = (ig[:, None] % G == np.arange(G)[None, :]).astype(np.float32)  # [ig, 8]
    gsumt = np.ascontiguousarray(gsum.T)                                   # [8, ig]

    nc = bacc.Bacc("TRN2", target_bir_lowering=False, debug=False,
                   enable_asserts=False, num_devices=NCORES, num_swdge_queues=4)

    TDT = BF16 if SEG_BF16 else F32
    t_table0 = nc.dram_tensor("table0", [H0, CG], TDT, kind="ExternalInput")
    t_table1 = nc.dram_tensor("table1", [N + 1 - H0, CG], TDT, kind="ExternalInput")
    t_idxa = nc.dram_tensor("idxa", [128, total_icols], I16, kind="ExternalInput")
    t_idxb = nc.dram_tensor("idxb", [128, total_icols], I16, kind="ExternalInput")
    t_sega = nc.dram_tensor("sega", [128, cols_pad], TDT, kind="ExternalInput")
    t_segb = nc.dram_tensor("segb", [128, cols_pad], TDT, kind="ExternalInput")
    t_xoh = nc.dram_tensor("xoh", [M, cols_pad], F32, kind="ExternalInput")
    t_cntb = nc.dram_tensor("cntb", [G, cols_pad], F32, kind="ExternalInput")
    t_lamq = nc.dram_tensor("lamq", [CG, C], F32, kind="ExternalInput")
    t_lamb = nc.dram_tensor("lamb", [CG, M], F32, kind="ExternalInput")
    t_gmask = nc.dram_tensor("gmask", [CG, CG], F32, kind="ExternalInput")
    t_gsum = nc.dram_tensor("gsum", [CG, G], F32, kind="ExternalInput")
    t_gsumt = nc.dram_tensor("gsumt", [G, CG], F32, kind="ExternalInput")
    t_post = nc.dram_tensor("post_t", [CG, cols_pad], F32, kind="ExternalOutput")
    t_like = nc.dram_tensor("like_t", [G, cols_pad], F32, kind="ExternalOutput")
    if DEBUG:
        t_dbg_qbig = nc.dram_tensor("dbg_qbig", [CG, CG], F32, kind="ExternalOutput")
        t_dbg_bm = nc.dram_tensor("dbg_bm", [M, CG], F32, kind="ExternalOutput")
        t_dbg_sumt = nc.dram_tensor("dbg_sumt", [CG, cols_pad], F32, kind="ExternalOutput")
        t_dbg_u = nc.dram_tensor("dbg_u", [CG, cols_pad], F32, kind="ExternalOutput")

    with tile.TileContext(nc) as tc, ExitStack() as ctx:
        consts = ctx.enter_context(tc.tile_pool(name="consts", bufs=1))
        setup = ctx.enter_context(tc.tile_pool(name="setup", bufs=1))
        msgs = ctx.enter_context(tc.tile_pool(name="msgs", bufs=3))
        io = ctx.enter_context(tc.tile_pool(name="io", bufs=2))
        work = ctx.enter_context(tc.tile_pool(name="work", bufs=2))
        psum_win = ctx.enter_context(tc.tile_pool(name="pwin", bufs=2, space="PSUM"))
        psum_ds = ctx.enter_context(tc.tile_pool(name="pds", bufs=1, space="PSUM"))

        def softmax_rows(lam_dram, f):
            """softmax along free dim of a [128, f] tile; returns SBUF tile."""
            lam = setup.tile([CG, f], F32, name=f"lam{f}")
            nc.sync.dma_start(lam[:], lam_dram.ap())
            mx = setup.tile([CG, 1], F32, name=f"mx{f}")
            nc.vector.tensor_reduce(mx[:], lam[:], axis=mybir.AxisListType.X,
                                    op=mybir.AluOpType.max, negate=True)
            e = setup.tile([CG, f], F32, name=f"e{f}")
            nc.scalar.activation(e[:], lam[:], mybir.ActivationFunctionType.Exp,
                                 bias=mx[:], scale=1.0)
            s = setup.tile([CG, 1], F32, name=f"s{f}")
            nc.vector.tensor_reduce(s[:], e[:], axis=mybir.AxisListType.X,
                                    op=mybir.AluOpType.add)
            r = setup.tile([CG, 1], F32, name=f"r{f}")
            nc.vector.reciprocal(r[:], s[:])
            out = setup.tile([CG, f], F32, name=f"sm{f}")
            nc.vector.tensor_scalar_mul(out[:], e[:], r[:])
            return out

        # ---- setup: Qbig [lg, ig] and Bm [m, ig] ----
        qs = softmax_rows(t_lamq, C)                     # [128, 16]
        gmask_t = consts.tile([CG, CG], F32)
        nc.sync.dma_start(gmask_t[:], t_gmask.ap())
        qexp = setup.tile([CG, CG], F32)
        nc.vector.tensor_copy(qexp[:], qs[:, :, None].to_broadcast([CG, C, G]))
        qbig = consts.tile([CG, CG], F32)
        nc.vector.tensor_tensor(qbig[:], qexp[:], gmask_t[:],
                                op=mybir.AluOpType.mult)

        bs = softmax_rows(t_lamb, M)                     # [128, 32]
        ident = setup.tile([CG, CG], F32)
        make_identity(nc, ident[:])
        bm_p = psum_ds.tile([M, CG], F32, space="PSUM")
        nc.tensor.transpose(bm_p[:], bs[:], ident[:])
        bm = consts.tile([M, CG], F32)
        nc.vector.tensor_copy(bm[:], bm_p[:])

        if DEBUG:
            nc.sync.dma_start(t_dbg_qbig.ap(), qbig[:])
            nc.sync.dma_start(t_dbg_bm.ap(), bm[:])
        gsum_t = consts.tile([CG, G], F32)
        nc.sync.dma_start(gsum_t[:], t_gsum.ap())
        gsumt_t = consts.tile([G, CG], F32)
        nc.sync.dma_start(gsumt_t[:], t_gsumt.ap())

        # ---- main loop over panels / chunks; gather calls every 16 chunks ----
        msg_tiles = {}
        for w in range(n_panels):
            pc = pp["panel_pc"][w]
            pw = pc * CW
            csl = slice(w * PANEL, w * PANEL + pw)

            sega_t = io.tile([128, pw], TDT, tag="sega")
            nc.sync.dma_start(sega_t[:], t_sega.ap()[:, csl])
            segb_t = io.tile([128, pw], TDT, tag="segb")
            nc.sync.dma_start(segb_t[:], t_segb.ap()[:, csl])
            xoh_t = io.tile([M, pw], F32, tag="xoh")
            nc.sync.dma_start(xoh_t[:], t_xoh.ap()[:, csl])
            cnt_t = io.tile([G, pw], F32, tag="cnt")
            nc.sync.dma_start(cnt_t[:], t_cntb.ap()[:, csl])

            win = psum_win.tile([CG, pw], F32, space="PSUM", tag="win")
            nc.vector.memset(win[:], 0.0)
            for k in range(0, pc, 2):
                t = w * CPP + k
                cl = t // (2 * CPC)
                if cl not in msg_tiles:
                    pr = pp["call_pairs"][cl]
                    nh = 128 * pr
                    isl = slice(int(icol_off[cl]), int(icol_off[cl + 1]))
                    idxa_t = io.tile([128, nh // 16], I16, tag="idxa")
                    nc.sync.dma_start(idxa_t[:], t_idxa.ap()[:, isl])
                    idxb_t = io.tile([128, nh // 16], I16, tag="idxb")
                    nc.sync.dma_start(idxb_t[:], t_idxb.ap()[:, isl])
                    msga = msgs.tile([128, pr, 128], TDT, tag="msga")
                    nc.gpsimd.dma_gather(msga[:], t_table0.ap(), idxa_t[:],
                                         nh, nh, CG, queue_num=(2 * cl) % 4)
                    msgb = msgs.tile([128, pr, 128], TDT, tag="msgb")
                    nc.gpsimd.dma_gather(msgb[:], t_table1.ap(), idxb_t[:],
                                         nh, nh, CG, queue_num=(2 * cl + 1) % 4)
                    msg_tiles = {cl: (msga, msgb)}
                msga, msgb = msg_tiles[cl]
                q0 = k * CW
                qn = min(2 * CW, pw - q0)
                s = (t // 2) % CPC
                nc.tensor.matmul(
                    win[:, q0:q0 + qn],
                    lhsT=msga[:, s, :],
                    rhs=sega_t[:, q0:q0 + qn],
                    start=False, stop=False, skip_group_check=True,
                )
                nc.tensor.matmul(
                    win[:, q0:q0 + qn],
                    lhsT=msgb[:, s, :],
                    rhs=segb_t[:, q0:q0 + qn],
                    start=False, stop=True, skip_group_check=True,
                )

            sumt = work.tile([CG, pw], F32, tag="sumt")
            nc.vector.tensor_copy(sumt[:], win[:])
            if DEBUG:
                nc.sync.dma_start(t_dbg_sumt.ap()[:, csl], sumt[:])

            def dsap(ap):
                return ap.bitcast(F32R) if DS_F32R else ap

            s_p = psum_ds.tile([CG, pw], F32, space="PSUM", tag="s")
            nc.tensor.matmul(s_p[:], lhsT=dsap(qbig[:]), rhs=dsap(sumt[:]),
                             start=True, stop=True)
            bx_p = psum_ds.tile([CG, pw], F32, space="PSUM", tag="bx")
            nc.tensor.matmul(bx_p[:], lhsT=dsap(bm[:]), rhs=dsap(xoh_t[:]),
                             start=True, stop=True)

            bx_s = work.tile([CG, pw], F32, tag="bxs")
            nc.vector.tensor_copy(bx_s[:], bx_p[:])
            u = work.tile([CG, pw], F32, tag="u")
            nc.vector.tensor_tensor(u[:], s_p[:], bx_s[:], op=mybir.AluOpType.mult)
            if DEBUG:
                nc.sync.dma_start(t_dbg_u.ap()[:, csl], u[:])

            tot_p = psum_ds.tile([G, pw], F32, space="PSUM", tag="tot")
            nc.tensor.matmul(tot_p[:], lhsT=dsap(gsum_t[:]), rhs=dsap(u[:]),
                             start=True, stop=True)

            rtot = work.tile([G, pw], F32, tag="rtot")
            nc.vector.reciprocal(rtot[:], tot_p[:])
            rb_p = psum_ds.tile([CG, pw], F32, space="PSUM", tag="rb")
            nc.tensor.matmul(rb_p[:], lhsT=dsap(gsumt_t[:]), rhs=dsap(rtot[:]),
                             start=True, stop=True)

            post_t = work.tile([CG, pw], F32, tag="post")
            nc.vector.tensor_tensor(post_t[:], u[:], rb_p[:], op=mybir.AluOpType.mult)
            nc.sync.dma_start(t_post.ap()[:, csl], post_t[:])

            totn = work.tile([G, pw], F32, tag="totn")
            nc.vector.tensor_tensor(totn[:], tot_p[:], cnt_t[:], op=mybir.AluOpType.mult)
            like_t = work.tile([G, pw], F32, tag="like")
            nc.scalar.activation(like_t[:], totn[:], mybir.ActivationFunctionType.Ln)
            nc.sync.dma_start(t_like.ap()[:, csl], like_t[:])

    nc.compile()

    in_maps = []
    for c in range(NCORES):
        in_maps.append({
            "table0": pp["table0"],
            "table1": pp["table1"],
            "idxa": pp["idxa"][c],
            "idxb": pp["idxb"][c],
            "sega": np.asarray(pp["sega"][c]),
            "segb": np.asarray(pp["segb"][c]),
            "xoh": np.asarray(pp["xoh"][c]),
            "cntb": pp["cntb"][c],
            "lamq": lamq_lg,
            "lamb": lamb_ig,
            "gmask": gmask,
            "gsum": gsum,
            "gsumt": gsumt,
        })
    return nc, in_maps


def _postprocess(pp, results):
    N, C, G = pp["N"], pp["C"], pp["G"]
    likelihood = np.empty((N, G), np.float32)
    posterior = np.empty((N, C, G), np.float32)
    for c in range(NCORES):
        colnode = pp["colnode"][c]
        valid = colnode >= 0
        nodes = colnode[valid]
        pt = results[c]["post_t"]   # [128, cols_pad]
        lt = results[c]["like_t"]   # [8, cols_pad]
        posterior[nodes] = np.ascontiguousarray(pt[:, valid].T).reshape(-1, C, G)
        likelihood[nodes] = lt[:, valid].T
    return likelihood, posterior


def kernel(x, edge_index, prev_h, lambda_Q, lambda_B):
    x = np.asarray(x)
    edge_index = np.asarray(edge_index)
    prev_h = np.asarray(prev_h)
    lambda_Q = np.asarray(lambda_Q)
    lambda_B = np.asarray(lambda_B)

    pp = _preprocess(x, edge_index, prev_h)
    nc, in_maps = _build_program(pp, lambda_Q, lambda_B)
    res = run_bass_kernel_spmd(nc, in_maps, list(range(NCORES)))
    return _postprocess(pp, res.results)
